# revision 31
# baseline (speedup 1.0000x reference)
"""Trainium2 Bass kernel for nn_Attention_52312701665770.

Reference computation (B=32, L=2048, D=C=1024):
    h        = einsum('bld,cd->blc', sequence, W_h) + b_h      # [B,L,C]
    energies = einsum('blc,bc->bl', h, query)                  # [B,L]
    scores   = softmax(where(arange(L) < lengths[:,None], energies, -inf))
    contexts = einsum('bl,bld->bd', scores, sequence)          # [B,D]
    returns (contexts, scores)

Algebraic rewrite used here:
    energies[b,l] = seq[b,l,:] . (W_h^T @ query[b])  + (b_h . query[b])
The bias term is constant per row -> softmax-invariant -> dropped entirely.
So per batch:  v = query @ W_h  (tiny matmul), one streaming pass over
sequence for the dot products, masked softmax, and a second (SBUF-resident)
pass for the weighted sum.  This turns a 137-GFLOP problem into a
256-MB-stream problem.

Sharding: data-parallel over batch: 4 batches per core x 8 cores.
W_h replicated.  Each core:
  - seq shard [4, 2048, 1024] streamed HBM->SBUF once (16 chunks/batch of
    [128 x 1024], both passes read the SBUF copy)
  - pass 1 (energies): DVE tensor_tensor_reduce (fused mul+row-sum) per chunk
    -> e[128, 16] per batch  (partition p, column c <-> l = c*128+p)
  - masked softmax: iota/length compare for the mask, cross-partition max/sum
    via PE transpose + ones-matmuls, exp (+row sums) fused on ScalarE
  - pass 2 (contexts): PE matmuls, scores column [128,1] stationary,
    seq chunks moving, PSUM accumulation over the 16 chunks
"""

import sys

if "/opt/trn_rl_repo" not in sys.path:
    sys.path.insert(0, "/opt/trn_rl_repo")

import numpy as np

B, L, D, C = 32, 2048, 1024, 1024
NCORES = 8
BPC = B // NCORES          # batches per core = 4
P = 128                    # partitions
NCH = L // P               # L chunks per batch = 16
KCH = C // P               # contraction chunks for v = q @ W_h = 8
NEG_BIG = -1.0e30

_cache = {}

# tunables (set before first kernel() call / _get_nc())
CFG = {
    "dma_group": 1,      # seq chunks per dma_start (1,2,4,8,16)
    "dma_rings": 1,      # 1: sync only; 2: alternate sync/scalar
    "seq_bufs": 32,      # seq pool slots (in units of group tiles)
    "repeat": 1,         # unrolled repetitions of the whole pipeline (benchmarking)
}


def _split_multi_waits(nc):
    """This walrus build accepts at most ONE sync-wait per instruction.
    Tile emits multi-wait instructions (notably the kernel-tail drain).
    Split: insert same-engine NoOps, each carrying one extra wait,
    immediately before the offending instruction.  Program order on the
    engine preserves the wait-before-execute semantics exactly."""
    from concourse import mybir

    n = 0
    for fn in nc.m.functions:
        for bb in fn.blocks:
            insts = bb.instructions
            i = 0
            while i < len(insts):
                inst = insts[i]
                si = inst.sync_info
                if si is not None and si.on_wait and len(si.on_wait) > 1:
                    extras = list(si.on_wait[:-1])
                    keep = [si.on_wait[-1]]
                    for w in extras:
                        nop = mybir.InstNoOp(name=f"I-wsplit{n}", ins=[], outs=[])
                        n += 1
                        nop.engine = inst.engine
                        nop.sync_info = mybir.SyncInfo(on_wait=[w], on_update=[])
                        nc.register_instruction(nop, overwrite=True)
                        insts.insert(i, nop)
                        i += 1
                    si.on_wait = keep
                i += 1
    return n


def _build_nc():
    """Build the per-core Bass module (SPMD: same program, different data)."""
    import concourse.bass as bass
    import concourse.tile as tile
    from concourse import mybir
    from concourse import bass_isa
    from concourse.masks import make_identity
    from contextlib import ExitStack

    f32 = mybir.dt.float32
    f32r = mybir.dt.float32r
    i32 = mybir.dt.int32
    OP = mybir.AluOpType
    AF = mybir.ActivationFunctionType
    AX = mybir.AxisListType

    nc = bass.Bass("TRN2", target_bir_lowering=False)

    v_dram = nc.dram_tensor("v_scratch", [BPC, D], f32)   # internal HBM bounce
    seq = nc.dram_tensor("seq", [BPC, L, D], f32, kind="ExternalInput")
    qT = nc.dram_tensor("qT", [C, BPC], f32, kind="ExternalInput")
    wh = nc.dram_tensor("wh", [C, D], f32, kind="ExternalInput")
    lens = nc.dram_tensor("lens", [1, BPC], i32, kind="ExternalInput")
    ctx_out = nc.dram_tensor("ctx_out", [BPC, D], f32, kind="ExternalOutput")
    sc_out = nc.dram_tensor("sc_out", [BPC, L], f32, kind="ExternalOutput")

    seq_v = seq.rearrange("b (c p) d -> b c p d", p=P)       # [4,16,128,1024]
    sc_v = sc_out.rearrange("b (c f) -> b c f", c=NCH)       # [4,16,128]

    with ExitStack() as ctx:
        tc = ctx.enter_context(tile.TileContext(nc))
        singles = ctx.enter_context(tc.tile_pool(name="singles", bufs=1))
        ps_big = ctx.enter_context(tc.tile_pool(name="ps_big", bufs=2, space="PSUM"))
        ps_sm = ctx.enter_context(tc.tile_pool(name="ps_sm", bufs=4, space="PSUM"))

        # ---- constants ----
        ident = singles.tile([P, P], f32)
        make_identity(nc, ident)

        iota_i = singles.tile([P, NCH], i32)
        nc.gpsimd.iota(iota_i, pattern=[[P, NCH]], base=0, channel_multiplier=1)
        iota_f = singles.tile([P, NCH], f32)
        nc.vector.tensor_copy(out=iota_f, in_=iota_i)

        # ---- lengths -> per-batch broadcast column [128,1] ----
        lens_sb = singles.tile([1, BPC], i32)
        nc.sync.dma_start(out=lens_sb, in_=lens[:, :])
        lens_f = singles.tile([1, BPC], f32)
        nc.vector.tensor_copy(out=lens_f, in_=lens_sb)
        ones_row = singles.tile([1, P], f32)
        nc.vector.memset(ones_row, 1.0)
        neg_row = singles.tile([1, P], f32)
        nc.vector.memset(neg_row, -1.0)
        ones_col = singles.tile([P, 1], f32)
        nc.vector.memset(ones_col, 1.0)
        lens_ps = ps_sm.tile([P, BPC], f32, tag="sps")
        nc.tensor.matmul(lens_ps, ones_row, lens_f)          # K=1 broadcast
        lens_bc = singles.tile([P, BPC], f32)
        nc.scalar.copy(out=lens_bc, in_=lens_ps)

        # additive masks for all batches, off the critical path:
        # mbias4[p, b, c] = (c*128+p >= lengths[b]) * -1e30
        mbias4 = singles.tile([P, BPC, NCH], f32)
        for b in range(BPC):
            nc.vector.tensor_scalar(
                out=mbias4[:, b, :],
                in0=iota_f,
                scalar1=lens_bc[:, b:b + 1],
                scalar2=NEG_BIG,
                op0=OP.is_ge,
                op1=OP.mult,
            )

        # ---- v = query @ W_h  -> per-batch broadcast rows vbc [128, D] ----
        # W_h lives in a transient pool so its 32 KB/partition is released
        # before the big seq pool is opened.
        with tc.tile_pool(name="whpool", bufs=1) as whpool:
            qT_sb = whpool.tile([P, KCH, BPC], f32)
            nc.sync.dma_start(out=qT_sb, in_=qT.rearrange("(k p) b -> p k b", p=P))
            wh_sb = whpool.tile([P, KCH, D], f32)
            nc.sync.dma_start(out=wh_sb, in_=wh.rearrange("(k p) d -> p k d", p=P))

            v_ps = ps_big.tile([BPC, D], f32, tag="bps")
            for k in range(KCH):
                for h in range(2):
                    nc.tensor.matmul(
                        v_ps[:, h * 512:(h + 1) * 512],
                        qT_sb[:, k, :],
                        wh_sb[:, k, h * 512:(h + 1) * 512],
                        start=(k == 0),
                        stop=(k == KCH - 1),
                    )
            v_sb = singles.tile([BPC, D], f32)
            nc.scalar.copy(out=v_sb, in_=v_ps)

        # bounce v through HBM, then DMA-replicate each row to 128 partitions
        nc.sync.dma_start(out=v_dram[:, :], in_=v_sb)
        vbc = singles.tile([P, BPC, D], f32)
        for b in range(BPC):
            nc.sync.dma_start(
                out=vbc[:, b, :],
                in_=v_dram[b:b + 1, :].to_broadcast([P, D]),
            )

        # ---- main per-batch pipeline ----
        grp = CFG["dma_group"]
        nring = CFG["dma_rings"]
        seqpool = ctx.enter_context(tc.tile_pool(name="seqp", bufs=CFG["seq_bufs"]))
        scrpool = ctx.enter_context(tc.tile_pool(name="scr", bufs=1))
        small = ctx.enter_context(tc.tile_pool(name="small", bufs=2))
        dma_i = 0
        for b in [bb for _ in range(CFG["repeat"]) for bb in range(BPC)]:
            seq_t = []
            e_sb = small.tile([P, NCH], f32, tag="e_sb")
            for c0 in range(0, NCH, grp):
                # tiles are f32r-typed (same bits as f32) so the pass-2
                # matmuls can run the full-rate FP32R PE path; DVE reads
                # them through f32 bitcast views.
                st = seqpool.tile([P, grp, D], f32r, tag="seq")
                eng = nc.sync if (dma_i % nring == 0) else nc.scalar
                dma_i += 1
                eng.dma_start(
                    out=st,
                    in_=seq_v[b, c0:c0 + grp].rearrange("c p d -> p c d").bitcast(f32r),
                )
                for j in range(grp):
                    seq_t.append(st[:, j, :])
                    scr = scrpool.tile([P, D], f32, tag="scr")
                    # fused multiply + free-dim sum on DVE:
                    # scr = seq * vbc, e column = sum(scr)
                    nc.vector.scalar_tensor_tensor(
                        out=scr,
                        in0=st[:, j, :].bitcast(f32),
                        scalar=0.0,
                        in1=vbc[:, b, :],
                        op0=OP.bypass,
                        op1=OP.mult,
                        accum_out=e_sb[:, c0 + j:c0 + j + 1],
                    )

            # additive mask (precomputed in prologue)
            e_m = small.tile([P, NCH], f32, tag="e_m")
            nc.vector.tensor_add(e_m, e_sb, mbias4[:, b, :])

            # row max (cross-partition): transpose + reduce, twice
            eT_ps = ps_sm.tile([NCH, P], f32, tag="sps")
            nc.tensor.transpose(eT_ps, e_m, ident)
            eT_sb = small.tile([NCH, P], f32, tag="eT_sb")
            nc.scalar.copy(out=eT_sb, in_=eT_ps)
            m16 = small.tile([NCH, 1], f32, tag="m16")
            nc.vector.tensor_reduce(out=m16, in_=eT_sb, axis=AX.X, op=OP.max)
            mT_ps = ps_sm.tile([1, NCH], f32, tag="sps")
            nc.tensor.transpose(mT_ps, m16, ident[0:NCH, 0:NCH])
            mT_sb = small.tile([1, NCH], f32, tag="mT_sb")
            nc.scalar.copy(out=mT_sb, in_=mT_ps)
            mrow = small.tile([1, 1], f32, tag="mrow")
            nc.vector.tensor_reduce(out=mrow, in_=mT_sb, axis=AX.X, op=OP.max)
            negm_ps = ps_sm.tile([P, 1], f32, tag="sps")
            nc.tensor.matmul(negm_ps, neg_row, mrow)         # -max, bcast
            negm = small.tile([P, 1], f32, tag="negm")
            nc.scalar.copy(out=negm, in_=negm_ps)

            # exp(e - max) with fused per-partition sums
            p_sb = small.tile([P, NCH], f32, tag="p_sb")
            s_part = small.tile([P, 1], f32, tag="s_part")
            nc.scalar.activation(
                out=p_sb, in_=e_m, func=AF.Exp, bias=negm, scale=1.0,
                accum_out=s_part,
            )
            s_ps = ps_sm.tile([1, 1], f32, tag="sps")
            nc.tensor.matmul(s_ps, ones_col, s_part)         # total sum
            s_sb = small.tile([1, 1], f32, tag="s_sb")
            nc.scalar.copy(out=s_sb, in_=s_ps)
            inv_sb = small.tile([1, 1], f32, tag="inv_sb")
            nc.vector.reciprocal(out=inv_sb, in_=s_sb)
            invbc_ps = ps_sm.tile([P, 1], f32, tag="sps")
            nc.tensor.matmul(invbc_ps, ones_row, inv_sb)     # 1/sum, bcast
            invbc = small.tile([P, 1], f32, tag="invbc")
            nc.scalar.copy(out=invbc, in_=invbc_ps)

            scores_sb = small.tile([P, NCH], f32r, tag="scores_sb")
            nc.vector.tensor_scalar_mul(scores_sb, p_sb, invbc)

            # pass 2: contexts[d] = sum_l scores[l] seq[l,d]
            # float32r = full-rate fp32 PE path (plain fp32 runs 4 cyc/col)
            ctx_ps = ps_big.tile([1, D], f32, tag="bps")
            for c in range(NCH):
                for h in range(2):
                    nc.tensor.matmul(
                        ctx_ps[:, h * 512:(h + 1) * 512],
                        scores_sb[:, c:c + 1],
                        seq_t[c][:, h * 512:(h + 1) * 512],
                        start=(c == 0),
                        stop=(c == NCH - 1),
                    )
            ctx_sb = small.tile([1, D], f32, tag="ctx_sb")
            nc.scalar.copy(out=ctx_sb, in_=ctx_ps)
            nc.sync.dma_start(out=ctx_out[b:b + 1, :], in_=ctx_sb)

            # scores out: transpose [128,16] -> [16,128] = row-major [2048]
            scT_ps = ps_sm.tile([NCH, P], f32, tag="sps")
            nc.tensor.transpose(scT_ps, scores_sb.bitcast(f32), ident)
            scT_sb = small.tile([NCH, P], f32, tag="scT_sb")
            nc.scalar.copy(out=scT_sb, in_=scT_ps)
            nc.sync.dma_start(out=sc_v[b], in_=scT_sb)

    _split_multi_waits(nc)
    return nc


def _get_nc():
    if "nc" not in _cache:
        _cache["nc"] = _build_nc()
    return _cache["nc"]


def kernel(sequence, query, lengths, W_h, b_h, trace=False, trace_kwargs=None):
    from concourse.bass_utils import run_bass_kernel_spmd

    sequence = np.ascontiguousarray(np.asarray(sequence, dtype=np.float32))
    query = np.asarray(query, dtype=np.float32)
    lengths = np.asarray(lengths, dtype=np.int32)
    W_h = np.ascontiguousarray(np.asarray(W_h, dtype=np.float32))
    # b_h shifts every energy in a row by the same constant (b_h . query[b]);
    # softmax is shift-invariant, so it cannot affect scores or contexts.

    nc = _get_nc()
    in_maps = []
    for core in range(NCORES):
        bs = slice(core * BPC, (core + 1) * BPC)
        in_maps.append({
            "seq": np.ascontiguousarray(sequence[bs]),
            "qT": np.ascontiguousarray(query[bs].T),
            "wh": W_h,
            "lens": np.ascontiguousarray(lengths[bs].reshape(1, BPC)),
        })

    kwargs = {}
    if trace:
        kwargs["trace"] = True
        if trace_kwargs:
            kwargs.update(trace_kwargs)
    res = run_bass_kernel_spmd(nc, in_maps, core_ids=list(range(NCORES)), **kwargs)
    _cache["last_results"] = res

    contexts = np.concatenate([r["ctx_out"] for r in res.results], axis=0)
    scores = np.concatenate([r["sc_out"] for r in res.results], axis=0)
    return contexts, scores


# revision 48
# speedup vs baseline: 1.2413x; 1.2413x over previous
"""Trainium2 Bass kernel for nn_Attention_52312701665770.

Reference computation (B=32, L=2048, D=C=1024):
    h        = einsum('bld,cd->blc', sequence, W_h) + b_h      # [B,L,C]
    energies = einsum('blc,bc->bl', h, query)                  # [B,L]
    scores   = softmax(where(arange(L) < lengths[:,None], energies, -inf))
    contexts = einsum('bl,bld->bd', scores, sequence)          # [B,D]
    returns (contexts, scores)

Algebraic rewrite used here:
    energies[b,l] = seq[b,l,:] . (W_h^T @ query[b])  + (b_h . query[b])
The bias term is constant per row -> softmax-invariant -> dropped entirely.
So per batch:  v = query @ W_h  (tiny matmul), one streaming pass over
sequence for the dot products, masked softmax, and a second (SBUF-resident)
pass for the weighted sum.  This turns a 137-GFLOP problem into a
256-MB-stream problem.

Sharding: data-parallel over batch: 4 batches per core x 8 cores.
W_h replicated.  Each core:
  - seq shard [4, 2048, 1024] streamed HBM->SBUF once (16 chunks/batch of
    [128 x 1024], both passes read the SBUF copy)
  - pass 1 (energies): DVE tensor_tensor_reduce (fused mul+row-sum) per chunk
    -> e[128, 16] per batch  (partition p, column c <-> l = c*128+p)
  - masked softmax: iota/length compare for the mask, cross-partition max/sum
    via PE transpose + ones-matmuls, exp (+row sums) fused on ScalarE
  - pass 2 (contexts): PE matmuls, scores column [128,1] stationary,
    seq chunks moving, PSUM accumulation over the 16 chunks
"""

import sys

if "/opt/trn_rl_repo" not in sys.path:
    sys.path.insert(0, "/opt/trn_rl_repo")

import numpy as np

B, L, D, C = 32, 2048, 1024, 1024
NCORES = 8
BPC = B // NCORES          # batches per core = 4
P = 128                    # partitions
NCH = L // P               # L chunks per batch = 16
KCH = C // P               # contraction chunks for v = q @ W_h = 8
NEG_BIG = -1.0e30

_cache = {}

# tunables (set before first kernel() call / _get_nc())
CFG = {
    "dma_group": 1,      # seq chunks per dma_start (1,2,4,8,16)
    "dma_rings": 2,      # 1: sync only; 2: alternate sync/scalar
    "seq_bufs": 36,      # seq pool slots (in units of group tiles)
    "repeat": 1,         # unrolled repetitions of the whole pipeline (benchmarking)
    "pool_stt": 0,       # chunks per batch (of 16) offloaded to GpSimd
}


def _split_multi_waits(nc):
    """This walrus build accepts at most ONE sync-wait per instruction.
    Tile emits multi-wait instructions (notably the kernel-tail drain).
    Split: insert same-engine NoOps, each carrying one extra wait,
    immediately before the offending instruction.  Program order on the
    engine preserves the wait-before-execute semantics exactly."""
    from concourse import mybir

    n = 0
    for fn in nc.m.functions:
        for bb in fn.blocks:
            insts = bb.instructions
            i = 0
            while i < len(insts):
                inst = insts[i]
                si = inst.sync_info
                if si is not None and si.on_wait and len(si.on_wait) > 1:
                    extras = list(si.on_wait[:-1])
                    keep = [si.on_wait[-1]]
                    for w in extras:
                        nop = mybir.InstNoOp(name=f"I-wsplit{n}", ins=[], outs=[])
                        n += 1
                        nop.engine = inst.engine
                        nop.sync_info = mybir.SyncInfo(on_wait=[w], on_update=[])
                        nc.register_instruction(nop, overwrite=True)
                        insts.insert(i, nop)
                        i += 1
                    si.on_wait = keep
                i += 1
    return n


def _build_nc():
    """Build the per-core Bass module (SPMD: same program, different data)."""
    import concourse.bass as bass
    import concourse.tile as tile
    from concourse import mybir
    from concourse import bass_isa
    from concourse.masks import make_identity
    from contextlib import ExitStack

    f32 = mybir.dt.float32
    f32r = mybir.dt.float32r
    i32 = mybir.dt.int32
    OP = mybir.AluOpType
    AF = mybir.ActivationFunctionType
    AX = mybir.AxisListType

    nc = bass.Bass("TRN2", target_bir_lowering=False)

    v_dram = nc.dram_tensor("v_scratch", [BPC, D], f32)   # internal HBM bounce
    seq = nc.dram_tensor("seq", [BPC, L, D], f32, kind="ExternalInput")
    qT = nc.dram_tensor("qT", [C, BPC], f32, kind="ExternalInput")
    wh = nc.dram_tensor("wh", [C, D], f32, kind="ExternalInput")
    lens = nc.dram_tensor("lens", [1, BPC], i32, kind="ExternalInput")
    ctx_out = nc.dram_tensor("ctx_out", [BPC, D], f32, kind="ExternalOutput")
    sc_out = nc.dram_tensor("sc_out", [BPC, L], f32, kind="ExternalOutput")

    seq_v = seq.rearrange("b (c p) d -> b c p d", p=P)       # [4,16,128,1024]
    sc_v = sc_out.rearrange("b (c f) -> b c f", c=NCH)       # [4,16,128]

    with ExitStack() as ctx:
        tc = ctx.enter_context(tile.TileContext(nc))
        singles = ctx.enter_context(tc.tile_pool(name="singles", bufs=1))
        ps_big = ctx.enter_context(tc.tile_pool(name="ps_big", bufs=2, space="PSUM"))
        ps_sm = ctx.enter_context(tc.tile_pool(name="ps_sm", bufs=4, space="PSUM"))

        # ---- constants ----
        ident = singles.tile([P, P], f32)
        make_identity(nc, ident)

        iota_i = singles.tile([P, NCH], i32)
        nc.gpsimd.iota(iota_i, pattern=[[P, NCH]], base=0, channel_multiplier=1)
        iota_f = singles.tile([P, NCH], f32)
        nc.vector.tensor_copy(out=iota_f, in_=iota_i)

        # ---- lengths -> per-batch broadcast column [128,1] ----
        lens_sb = singles.tile([1, BPC], i32)
        nc.sync.dma_start(out=lens_sb, in_=lens[:, :])
        lens_f = singles.tile([1, BPC], f32)
        nc.vector.tensor_copy(out=lens_f, in_=lens_sb)
        ones_row = singles.tile([1, P], f32)
        nc.vector.memset(ones_row, 1.0)
        neg_row = singles.tile([1, P], f32)
        nc.vector.memset(neg_row, -1.0)
        ones_col = singles.tile([P, 1], f32)
        nc.vector.memset(ones_col, 1.0)
        lens_ps = ps_sm.tile([P, BPC], f32, tag="sps")
        nc.tensor.matmul(lens_ps, ones_row, lens_f)          # K=1 broadcast
        lens_bc = singles.tile([P, BPC], f32)
        nc.scalar.copy(out=lens_bc, in_=lens_ps)

        # additive masks for all batches, off the critical path:
        # mbias4[p, b, c] = (c*128+p >= lengths[b]) * -1e30
        mbias4 = singles.tile([P, BPC, NCH], f32)
        for b in range(BPC):
            nc.vector.tensor_scalar(
                out=mbias4[:, b, :],
                in0=iota_f,
                scalar1=lens_bc[:, b:b + 1],
                scalar2=NEG_BIG,
                op0=OP.is_ge,
                op1=OP.mult,
            )

        # PE warmup: ~3.4us of dummy matmuls releases the HAM clock throttle
        # (1.2 -> 2.4 GHz) before the V matmuls arrive.
        warm = singles.tile([P, 512], f32)
        nc.gpsimd.memset(warm, 1.0)
        warm_ps = ps_sm.tile([1, 512], f32, tag="sps")
        for _ in range(5):
            nc.tensor.matmul(warm_ps, warm[:, 0:1].bitcast(f32r),
                             warm.bitcast(f32r), start=True, stop=True)

        # ---- v = query @ W_h  -> per-batch broadcast rows vbc [128, D] ----
        # W_h is DMA'd in per-k chunks (alternating HWDGE rings) so each V
        # matmul starts as soon as its chunk lands; the pool is transient so
        # its 32 KB/partition is released before the big seq pool opens.
        with tc.tile_pool(name="whpool", bufs=1) as whpool:
            qT_sb = whpool.tile([P, KCH, BPC], f32)
            nc.sync.dma_start(out=qT_sb, in_=qT.rearrange("(k p) b -> p k b", p=P))
            wh_sb = whpool.tile([P, KCH, D], f32)
            wh_v = wh.rearrange("(k p) d -> p k d", p=P)
            for k in range(KCH):
                eng = nc.sync if k % 2 == 0 else nc.scalar
                eng.dma_start(out=wh_sb[:, k, :], in_=wh_v[:, k, :])

            v_ps = ps_big.tile([BPC, D], f32, tag="bps")
            for k in range(KCH):
                for h in range(2):
                    nc.tensor.matmul(
                        v_ps[:, h * 512:(h + 1) * 512],
                        qT_sb[:, k, :],
                        wh_sb[:, k, h * 512:(h + 1) * 512],
                        start=(k == 0),
                        stop=(k == KCH - 1),
                    )
            v_sb = singles.tile([BPC, D], f32)
            nc.scalar.copy(out=v_sb, in_=v_ps)

        # broadcast v rows to all 128 partitions with one-hot selector
        # matmuls (PE is warm and otherwise idle here; avoids queueing the
        # broadcast behind the seq DMA streams)
        sel = singles.tile([BPC, BPC, P], f32)
        nc.gpsimd.memset(sel, 0.0)
        # predicate (k - b) != 0 -> keep 0, else fill 1  => sel[k,b,:]=(k==b)
        nc.gpsimd.affine_select(
            out=sel, in_=sel, compare_op=OP.not_equal, fill=1.0,
            base=0, pattern=[[-1, BPC], [0, P]], channel_multiplier=1,
        )
        vbc = singles.tile([P, BPC, D], f32)
        for b in range(BPC):
            vbc_ps = ps_big.tile([P, D], f32, tag="bps")
            for h in range(2):
                nc.tensor.matmul(
                    vbc_ps[:, h * 512:(h + 1) * 512],
                    sel[:, b, :],
                    v_sb[:, h * 512:(h + 1) * 512],
                )
            nc.scalar.copy(out=vbc[:, b, :], in_=vbc_ps)

        # ---- main per-batch pipeline (software-pipelined emission:
        #      batch b's loads+pass1 are emitted before batch b-1's
        #      softmax+pass2 so the DVE stream never stalls on the
        #      cross-engine softmax round trips) ----
        grp = CFG["dma_group"]
        nring = CFG["dma_rings"]
        seqpool = ctx.enter_context(tc.tile_pool(name="seqp", bufs=CFG["seq_bufs"]))
        scrpool = ctx.enter_context(tc.tile_pool(name="scr", bufs=1))
        small = ctx.enter_context(tc.tile_pool(name="small", bufs=2))
        state = {}
        dma_i = 0

        def emit_load_pass1(b):
            nonlocal dma_i
            seq_t = []
            e_sb = small.tile([P, NCH], f32, tag="e_sb")
            for c0 in range(0, NCH, grp):
                # tiles are f32r-typed (same bits as f32) so the pass-2
                # matmuls can run the full-rate FP32R PE path; DVE reads
                # them through f32 bitcast views.
                st = seqpool.tile([P, grp, D], f32r, tag="seq")
                eng = nc.sync if (dma_i % nring == 0) else nc.scalar
                dma_i += 1
                eng.dma_start(
                    out=st,
                    in_=seq_v[b, c0:c0 + grp].rearrange("c p d -> p c d").bitcast(f32r),
                )
                for j in range(grp):
                    c = c0 + j
                    seq_t.append(st[:, j, :])
                    # fused multiply + free-dim sum (scr = seq * vbc,
                    # e column = sum(scr)); most chunks on DVE, some on the
                    # otherwise-idle GpSimd engine
                    eng_v = nc.vector   # walrus rejects TensorScalarPtr on Pool
                    scr = scrpool.tile([P, D], f32, tag="scr")
                    eng_v.scalar_tensor_tensor(
                        out=scr,
                        in0=st[:, j, :].bitcast(f32),
                        scalar=0.0,
                        in1=vbc[:, b, :],
                        op0=OP.bypass,
                        op1=OP.mult,
                        accum_out=e_sb[:, c:c + 1],
                    )

            state[b] = (seq_t, e_sb)

        def emit_softmax_pass2(b):
            seq_t, e_sb = state.pop(b)
            # additive mask (precomputed in prologue)
            e_m = small.tile([P, NCH], f32, tag="e_m")
            nc.vector.tensor_add(e_m, e_sb, mbias4[:, b, :])

            # row max (cross-partition): transpose + reduce, twice.
            # DVE reduces read straight from PSUM (skip the ACT copies).
            eT_ps = ps_sm.tile([NCH, P], f32, tag="sps")
            nc.tensor.transpose(eT_ps, e_m, ident)
            m16 = small.tile([NCH, 1], f32, tag="m16")
            nc.vector.tensor_reduce(out=m16, in_=eT_ps, axis=AX.X, op=OP.max)
            mT_ps = ps_sm.tile([1, NCH], f32, tag="sps")
            nc.tensor.transpose(mT_ps, m16, ident[0:NCH, 0:NCH])
            mrow = small.tile([1, 1], f32, tag="mrow")
            nc.vector.tensor_reduce(out=mrow, in_=mT_ps, axis=AX.X, op=OP.max)
            negm_ps = ps_sm.tile([P, 1], f32, tag="sps")
            nc.tensor.matmul(negm_ps, neg_row, mrow)         # -max, bcast
            negm = small.tile([P, 1], f32, tag="negm")
            nc.scalar.copy(out=negm, in_=negm_ps)

            # exp(e - max) with fused per-partition sums; p is written as
            # f32r so pass-2 can consume it UNNORMALIZED right away — the
            # 1/sum scale is applied to the [1, D] result afterwards,
            # taking the sum/reciprocal chain off the pass-2 critical path.
            p_sb = small.tile([P, NCH], f32r, tag="p_sb")
            s_part = small.tile([P, 1], f32, tag="s_part")
            nc.scalar.activation(
                out=p_sb, in_=e_m, func=AF.Exp, bias=negm, scale=1.0,
                accum_out=s_part,
            )
            s_ps = ps_sm.tile([1, 1], f32, tag="sps")
            nc.tensor.matmul(s_ps, ones_col, s_part)         # total sum
            inv_sb = small.tile([1, 1], f32, tag="inv_sb")
            nc.vector.reciprocal(out=inv_sb, in_=s_ps)
            invbc_ps = ps_sm.tile([P, 1], f32, tag="sps")
            nc.tensor.matmul(invbc_ps, ones_row, inv_sb)     # 1/sum, bcast

            # pass 2 (unnormalized): ctx_raw[d] = sum_l p[l] seq[l,d]
            # float32r = full-rate fp32 PE path (plain fp32 runs 4 cyc/col)
            ctx_ps = ps_big.tile([1, D], f32, tag="bps")
            for c in range(NCH):
                for h in range(2):
                    nc.tensor.matmul(
                        ctx_ps[:, h * 512:(h + 1) * 512],
                        p_sb[:, c:c + 1],
                        seq_t[c][:, h * 512:(h + 1) * 512],
                        start=(c == 0),
                        stop=(c == NCH - 1),
                    )
            ctx_sb = small.tile([1, D], f32, tag="ctx_sb")
            nc.scalar.mul(ctx_sb, ctx_ps, inv_sb)   # normalize on idle ACT
            nc.sync.dma_start(out=ctx_out[b:b + 1, :], in_=ctx_sb)

            # scores out: normalize then transpose [128,16] -> [16,128]
            scores_sb = small.tile([P, NCH], f32, tag="scores_sb")
            nc.vector.tensor_scalar_mul(scores_sb, p_sb.bitcast(f32),
                                        invbc_ps[:, 0:1])
            scT_ps = ps_sm.tile([NCH, P], f32, tag="sps")
            nc.tensor.transpose(scT_ps, scores_sb, ident)
            scT_sb = small.tile([NCH, P], f32, tag="scT_sb")
            nc.scalar.copy(out=scT_sb, in_=scT_ps)
            nc.scalar.dma_start(out=sc_v[b], in_=scT_sb)

        batches = [bb for _ in range(CFG["repeat"]) for bb in range(BPC)]
        prev = None
        for b in batches:
            emit_load_pass1(b)
            if prev is not None:
                emit_softmax_pass2(prev)
            prev = b
        emit_softmax_pass2(prev)

    _split_multi_waits(nc)
    return nc


def _get_nc():
    if "nc" not in _cache:
        _cache["nc"] = _build_nc()
    return _cache["nc"]


def kernel(sequence, query, lengths, W_h, b_h, trace=False, trace_kwargs=None):
    from concourse.bass_utils import run_bass_kernel_spmd

    sequence = np.ascontiguousarray(np.asarray(sequence, dtype=np.float32))
    query = np.asarray(query, dtype=np.float32)
    lengths = np.asarray(lengths, dtype=np.int32)
    W_h = np.ascontiguousarray(np.asarray(W_h, dtype=np.float32))
    # b_h shifts every energy in a row by the same constant (b_h . query[b]);
    # softmax is shift-invariant, so it cannot affect scores or contexts.

    nc = _get_nc()
    in_maps = []
    for core in range(NCORES):
        bs = slice(core * BPC, (core + 1) * BPC)
        in_maps.append({
            "seq": np.ascontiguousarray(sequence[bs]),
            "qT": np.ascontiguousarray(query[bs].T),
            "wh": W_h,
            "lens": np.ascontiguousarray(lengths[bs].reshape(1, BPC)),
        })

    kwargs = {}
    if trace:
        kwargs["trace"] = True
        if trace_kwargs:
            kwargs.update(trace_kwargs)
    res = run_bass_kernel_spmd(nc, in_maps, core_ids=list(range(NCORES)), **kwargs)
    _cache["last_results"] = res

    contexts = np.concatenate([r["ctx_out"] for r in res.results], axis=0)
    scores = np.concatenate([r["sc_out"] for r in res.results], axis=0)
    return contexts, scores
